# revision 49
# baseline (speedup 1.0000x reference)
"""Trainium2 Bass kernel for nn_Disc_53515292508892 (ragged_sequence).

Computes: src-GRU (H=1024) over ragged [128,64] token batch -> final hidden,
tgt-GRU seeded with it, then a 2-layer head -> logits [64, 2].
(The reference's ref-encoder outputs are computed then deleted -- dead code --
so they are skipped.)

Sharding: data-parallel over batch, B=64 -> 8 sequences per NeuronCore,
GRU weights replicated, no inter-core communication.

Per-core layout (fp16 compute, fp32 PSUM):
  - hidden state kept twice:
      h_str  [128, 256] : partition 32j+b (j = col-group, b = batch lane),
                          free = hidden unit within group (256 each)
      hT_buf [128, 256] : transposed (hidden-on-partition) = matmul lhsT
  - Whh reordered so col-group j holds (r_j | (1-z)_j | n_j) gate columns;
    recurrent matmul runs 4-way col-tiled via tile_position=(0,32j), so the
    four 768-column weight streams flow through the PE concurrently.
  - xW = x @ Wih.T (+ folded biases) precomputed on device into DRAM rows
    [(t*8+b), 3072], streamed back 48KB/step, double-buffered.

Perf ledger (HW exec, NTFF-profiled; this file ships the best config):
  2.758 ms  first correct version (single shared PSUM accumulator)
  2.10-2.14 ms  SHIPPED: split PSUM rz/n accumulators (sigma waits only on
                the 32 rz-matmuls; also keeps the PE gap under control)
  3.043 ms  REJECTED two-chain batch interleave -- the weight stream is
            batch-independent, so splitting the batch doubles PE work
  2.44-2.45 ms  REJECTED h-update refactor (p=m*z', h=p*n+(1-p)*h) +
                keep-warm dummy transposes + phase-1 dripping
  2.353 ms  REJECTED sigma r/z-half split + ACT/DVE parallel copies
  2.348 ms  REJECTED evens-first k-order + parallel copies alone
  (compile fail)  manual then_inc/_wait_ge to delay n-matmuls into the
                  sigma window: walrus setupSyncUpdate rejects an extra
                  sem update on a Tile-managed DVE op
Lesson: ops here are [128, 256-512] -- fixed per-op + semaphore-hop costs
(~150-250ns) dominate, so adding/splitting ops to shorten chain latency
loses; only wholesale work removal wins. Remaining known headroom
(~0.5 ms): HAM re-throttles the PE to 1.2 GHz during each step's ~3 us
gate window; needs dense real PE work in-window (half-hidden-chunk
pipeline) without net-new chain ops.
"""

import sys
import functools

sys.path.insert(0, "/opt/trn_rl_repo")

import numpy as np
import concourse.mybir as mybir
from concourse import bacc, tile
from concourse.bass_utils import run_bass_kernel_spmd

f16 = mybir.dt.float16
f32 = mybir.dt.float32
AO = mybir.AluOpType
AF = mybir.ActivationFunctionType

V, D, H = 32000, 512, 1024
T = 128          # steps per GRU (T_SRC = T_TGT = 128)
BL = 8           # batch per core
NCORES = 8
NG = 4           # col-tile groups
GW = 768         # gate columns per group (256 r | 256 z' | 256 n)
KT = H // 128    # 8 k-tiles over hidden
KD = D // 128    # 4 k-tiles over embedding dim


# ----------------------------------------------------------------------------
# host-side weight/layout prep
# ----------------------------------------------------------------------------

def _gate_perm():
    """perm[c] = original Whh/Wih row for reordered gate column c.
    Gate-major layout [r 1024 | n 1024 | z' 1024]; within a gate, unit
    256j+u belongs to quadrant group j. One DMA each for the per-step r/z
    blocks; sign = -1 for z' columns (z' = 1-z = sigmoid(-pre_z))."""
    c = np.arange(3072)
    gate = c // 1024
    unit = c % 1024
    row = np.where(gate == 0, unit,
                   np.where(gate == 1, 2048 + unit, 1024 + unit))
    sign = np.where(gate == 2, -1.0, 1.0).astype(np.float32)
    is_n = gate == 1
    return row, sign, is_n


def _prep_shared(inputs, n_steps):
    """Core-independent tensors (weights, biases, identity)."""
    row, sign, is_n = _gate_perm()
    out = {}
    for g, wih, whh, bih, bhh in (
        ("src", inputs["src_Wih"], inputs["src_Whh"], inputs["src_bih"], inputs["src_bhh"]),
        ("tgt", inputs["tgt_Wih"], inputs["tgt_Whh"], inputs["tgt_bih"], inputs["tgt_bhh"]),
    ):
        whh_a = (whh[row] * sign[:, None]).T.astype(np.float16)          # [1024, 3072]
        out[f"whh_{g}"] = np.ascontiguousarray(whh_a.reshape(KT, 128, 3072))
        wih_a = (wih[row] * sign[:, None]).T.astype(np.float16)          # [512, 3072]
        out[f"wih_{g}"] = np.ascontiguousarray(wih_a.reshape(KD, 128, 3072))
        bias_vec = sign * bih[row] + sign * np.where(is_n, 0.0, bhh[row])
        out[f"bias_{g}"] = np.broadcast_to(
            bias_vec.astype(np.float16), (128, 3072)).copy()
        # bhh for n-gate on one partition: col 256j+u = bhh[2048+256j+u];
        # streamed into PSUM via a k=1 ones-matmul preload
        bhnr = np.zeros((1, 1024), np.float16)
        for j in range(NG):
            bhnr[0, 256 * j:256 * (j + 1)] = bhh[2048 + 256 * j:2048 + 256 * (j + 1)].astype(np.float16)
        out[f"bhnr_{g}"] = bhnr
    p1 = inputs["p1_W"].T.reshape(KT, 128, 64).transpose(1, 0, 2).reshape(128, KT * 64)
    out["p1T"] = p1.astype(np.float16)
    out["p1b"] = np.broadcast_to(inputs["p1_b"].astype(np.float16), (128, 64)).copy()
    out["p2T"] = inputs["p2_W"].T.astype(np.float16)                      # [64, 2]
    out["p2b"] = np.broadcast_to(inputs["p2_b"].astype(np.float32), (128, 2)).copy()
    out["ident"] = np.eye(128, dtype=np.float16)
    out["ones1"] = np.ones((1, BL), np.float16)
    return out


def _prep_core(inputs, emb16, core, n_steps):
    """Per-core tensors: gathered/transposed token embeddings and masks."""
    sl = slice(BL * core, BL * (core + 1))
    out = {}
    for g, ids_key, len_key in (("src", "src", "src_lengths"),
                                ("tgt", "tgt", "tgt_lengths")):
        ids = np.asarray(inputs[ids_key])[:n_steps, sl]                   # [T, 8]
        x = emb16[ids]                                                    # [T, 8, 512]
        out[f"xT_{g}"] = np.ascontiguousarray(
            x.transpose(2, 0, 1).reshape(KD, 128, n_steps * BL))
    # per-partition z'-gate mask bias for phase 1: xw row r = t*BL + b of
    # m-strip mi sits on partition p = r - 128*mi; -60 on z' cols where
    # t >= len freezes h (sigmoid(-60+pre) ~ 0)
    n_mstrip = (n_steps * BL + 127) // 128
    mb = np.zeros((128, 2 * n_mstrip), np.float32)
    for gi, len_key in enumerate(("src_lengths", "tgt_lengths")):
        ln = np.asarray(inputs[len_key])[sl]                              # [8]
        r = np.arange(n_steps * BL)
        masked = (r // BL) >= ln[r % BL]                                  # [T*BL]
        mb[:, gi * n_mstrip:(gi + 1) * n_mstrip] = np.where(
            masked, -60.0, 0.0).reshape(n_mstrip, 128).T
    out["maskbias"] = mb
    return out


# ----------------------------------------------------------------------------
# device program
# ----------------------------------------------------------------------------

def build_program(n_steps=T, debug=False):
    nc = bacc.Bacc("TRN2", target_bir_lowering=False, debug=False,
                   num_devices=NCORES)
    TB = n_steps * BL

    dp = nc.declare_dram_parameter
    d_xT = {g: dp(f"xT_{g}", [KD, 128, TB], f16, isOutput=False) for g in ("src", "tgt")}
    d_whh = {g: dp(f"whh_{g}", [KT, 128, 3072], f16, isOutput=False) for g in ("src", "tgt")}
    d_wih = {g: dp(f"wih_{g}", [KD, 128, 3072], f16, isOutput=False) for g in ("src", "tgt")}
    d_bias = {g: dp(f"bias_{g}", [128, 3072], f16, isOutput=False) for g in ("src", "tgt")}
    d_bhnr = {g: dp(f"bhnr_{g}", [1, 1024], f16, isOutput=False) for g in ("src", "tgt")}
    n_mstrip_d = (TB + 127) // 128
    d_maskbias = dp("maskbias", [128, 2 * n_mstrip_d], f32, isOutput=False)
    d_ones1 = dp("ones1", [1, BL], f16, isOutput=False)
    d_p1T = dp("p1T", [128, KT * 64], f16, isOutput=False)
    d_p1b = dp("p1b", [128, 64], f16, isOutput=False)
    d_p2T = dp("p2T", [64, 2], f16, isOutput=False)
    d_p2b = dp("p2b", [128, 2], f32, isOutput=False)
    d_ident = dp("ident", [128, 128], f16, isOutput=False)
    d_logits = dp("logits", [BL, 2], f32, isOutput=True)
    if debug:
        d_dbg_h = dp("dbg_h", [128, 256], f16, isOutput=True)
        d_dbg_xw = {g: dp(f"dbg_xw_{g}", [TB, 3072], f16, isOutput=True)
                    for g in ("src", "tgt")}

    with tile.TileContext(nc) as tc:
        with tc.tile_pool(name="const", bufs=1) as cpool, \
             tc.tile_pool(name="work", bufs=2) as wpool, \
             tc.tile_pool(name="xwload", bufs=4) as xwpool, \
             tc.tile_pool(name="p1ev", bufs=4) as evpool, \
             tc.tile_pool(name="psum", bufs=2, space="PSUM") as psum, \
             tc.tile_pool(name="dram", bufs=1, space="DRAM") as dram:

            # ---- resident constants -------------------------------------
            whh_sb, xT_sb, bias_sb, bhhn_sb = {}, {}, {}, {}
            for g in ("src", "tgt"):
                whh_sb[g] = cpool.tile([128, KT * 3072], f16, tag=f"whh_{g}", name=f"whh_{g}")
                for k in range(KT):
                    nc.sync.dma_start(whh_sb[g][:, 3072 * k:3072 * (k + 1)], d_whh[g][k])
                xT_sb[g] = cpool.tile([128, KD * TB], f16, tag=f"xT_{g}", name=f"xT_{g}")
                for k in range(KD):
                    nc.sync.dma_start(xT_sb[g][:, TB * k:TB * (k + 1)], d_xT[g][k])
                bias_sb[g] = cpool.tile([128, 3072], f16, tag=f"bias_{g}", name=f"biassb_{g}")
                nc.sync.dma_start(bias_sb[g][:], d_bias[g][:])
                bhhn_sb[g] = cpool.tile([1, 1024], f16, tag=f"bhnr_{g}", name=f"bhnrsb_{g}")
                nc.sync.dma_start(bhhn_sb[g][:], d_bhnr[g][:])
            mb_sb = cpool.tile([128, 2 * n_mstrip_d], f32, tag="maskbias")
            nc.sync.dma_start(mb_sb[:], d_maskbias[:])
            ident_sb = cpool.tile([128, 128], f16, tag="ident")
            nc.sync.dma_start(ident_sb[:], d_ident[:])
            ones1_sb = cpool.tile([1, BL], f16, tag="ones1")
            nc.sync.dma_start(ones1_sb[:], d_ones1[:])
            p1T_sb = cpool.tile([128, KT * 64], f16, tag="p1T")
            nc.sync.dma_start(p1T_sb[:], d_p1T[:])
            p1b_sb = cpool.tile([128, 64], f16, tag="p1b")
            nc.sync.dma_start(p1b_sb[:], d_p1b[:])
            p2T_sb = cpool.tile([64, 2], f16, tag="p2T")
            nc.sync.dma_start(p2T_sb[:], d_p2T[:])
            p2b_sb = cpool.tile([128, 2], f32, tag="p2b")
            nc.sync.dma_start(p2b_sb[:], d_p2b[:])

            # ---- phase 1 (as a generator of work units so tgt can be
            # interleaved into the src recurrence to fill PE gaps) ---------
            xw_dram = {}
            for g in ("src", "tgt"):
                xw_dram[g] = dram.tile([TB, 3072], f16, tag=f"xw_{g}", name=f"xwdram_{g}")
            n_mstrip = (TB + 127) // 128

            def phase1_units(g):
                for ch in range(6):
                    wihs = []
                    for kd in range(KD):
                        wt = evpool.tile([128, 512], f16, tag="wih_s", name="wih_s",
                                         bufs=8)
                        nc.sync.dma_start(
                            wt[:], d_wih[g][kd, :, 512 * ch:512 * (ch + 1)])
                        wihs.append(wt)
                    for mi in range(n_mstrip):
                        m0 = 128 * mi
                        msz = min(128, TB - m0)
                        ps = psum.tile([128, 512], f32, tag="p1", name="p1ps")
                        for kd in range(KD):
                            nc.tensor.matmul(
                                ps[0:msz, :],
                                xT_sb[g][:, TB * kd + m0: TB * kd + m0 + msz],
                                wihs[kd][:],
                                start=(kd == 0), stop=(kd == KD - 1),
                            )
                        ev = evpool.tile([128, 512], f16, tag="ev", name="ev")
                        if ch < 4:
                            nc.vector.tensor_add(
                                ev[0:msz, :], ps[0:msz, :],
                                bias_sb[g][0:msz, 512 * ch:512 * (ch + 1)])
                        else:
                            # chunks 4-5 are all z' columns: also add the
                            # per-row -60 freeze bias (masks h past length)
                            gi = 0 if g == "src" else 1
                            nc.vector.scalar_tensor_tensor(
                                ev[0:msz, :], ps[0:msz, :],
                                mb_sb[0:msz, gi * n_mstrip_d + mi:
                                      gi * n_mstrip_d + mi + 1],
                                bias_sb[g][0:msz, 512 * ch:512 * (ch + 1)],
                                AO.add, AO.add)
                        nc.sync.dma_start(
                            xw_dram[g][m0:m0 + msz, 512 * ch:512 * (ch + 1)],
                            ev[0:msz, :])
                        yield

            for g in ("src", "tgt"):
                for _ in phase1_units(g):
                    pass

            # ---- recurrence ---------------------------------------------
            h_str = wpool.tile([128, 256], f16, tag="h_str")
            hT = wpool.tile([128, 256], f16, tag="hT")
            nc.vector.memset(h_str[:], 0.0)
            nc.vector.memset(hT[:], 0.0)


            def stage_in(step):
                """DMA xw + psum accumulators + PE preloads for `step`.
                Called one step ahead so the tiny preload matmuls (8/1-row
                stationaries: high busy, ~6% array power) land in the PE's
                post-stream idle window -- real keep-warm work for the HAM
                clock gate -- and the xw adds never touch the gate chain."""
                g = "src" if step < n_steps else "tgt"
                r0 = (step % n_steps) * BL
                # xw r/z blocks at base partition 0 (stationary base partition
                # must equal the PE tile row); xn in strip layout for sn
                xw_rb = xwpool.tile([BL, 1024], f16, tag="xw_rb", name="xw_rb")
                xw_zb = xwpool.tile([BL, 1024], f16, tag="xw_zb", name="xw_zb")
                xw_n = xwpool.tile([128, 256], f16, tag="xw_n", name="xw_n")
                nc.sync.dma_start(xw_rb[:], xw_dram[g][r0:r0 + BL, 0:1024])
                nc.sync.dma_start(xw_zb[:], xw_dram[g][r0:r0 + BL, 2048:3072])
                for j in range(NG):
                    nc.sync.dma_start(
                        xw_n[32 * j:32 * j + BL, :],
                        xw_dram[g][r0:r0 + BL, 1024 + 256 * j:1024 + 256 * (j + 1)])
                pmm_r = psum.tile([128, 256], f32, tag="mm_r", name="pmm_r",
                                  bufs=1)
                pmm_n = psum.tile([128, 256], f32, tag="mm_n", name="pmm_n",
                                  bufs=1)
                pmm_z = psum.tile([128, 256], f32, tag="mm_z", name="pmm_z",
                                  bufs=1)
                for j in range(NG):
                    nc.tensor.matmul(
                        pmm_r[32 * j:32 * j + BL, :],
                        ident_sb[0:BL, 0:BL],
                        xw_rb[:, 256 * j:256 * (j + 1)],
                        start=True, stop=False, tile_position=(0, 32 * j))
                for j in range(NG):
                    nc.tensor.matmul(
                        pmm_n[32 * j:32 * j + BL, :],
                        ones1_sb[0:1, :],
                        bhhn_sb[g][0:1, 256 * j:256 * (j + 1)],
                        start=True, stop=False, tile_position=(0, 32 * j))
                for j in range(NG):
                    nc.tensor.matmul(
                        pmm_z[32 * j:32 * j + BL, :],
                        ident_sb[0:BL, 0:BL],
                        xw_zb[:, 256 * j:256 * (j + 1)],
                        start=True, stop=False, tile_position=(0, 32 * j))
                return g, pmm_r, pmm_n, pmm_z, xw_n

            cur = stage_in(0)
            for step in range(2 * n_steps):
                g, pmm_r, pmm_n, pmm_z, xw_n = cur

                # three 256-col weight streams r -> n -> z: sig_r fires after
                # the first, the gate chain overlaps the rest. Even k-tiles
                # first: they read hT cols 0:128, produced by the first half
                # of the previous step's split transpose/hT-update.
                for c0, dst in ((0, pmm_r), (1024, pmm_n), (2048, pmm_z)):
                    for ki, k in enumerate((0, 2, 4, 6, 1, 3, 5, 7)):
                        coff = 128 * (k % 2) + 32 * (k // 2)
                        lhsT = hT[:, coff:coff + BL]
                        for j in range(NG):
                            nc.tensor.matmul(
                                dst[32 * j:32 * j + BL, :],
                                lhsT,
                                whh_sb[g][:, 3072 * k + c0 + 256 * j:
                                        3072 * k + c0 + 256 * (j + 1)],
                                start=False, stop=(ki == KT - 1),
                                tile_position=(0, 32 * j),
                            )

                # stage the next step now: its DMAs + PE preloads fill the
                # idle window between the z-stream and the transposes
                nxt = stage_in(step + 1) if step + 1 < 2 * n_steps else None

                # gates (strip view [128, *]; only partitions 32j+b<8 valid);
                # sigmoids read PSUM directly
                rz = wpool.tile([128, 512], f16, tag="rz")
                nc.scalar.activation(rz[:, 0:256], pmm_r[:], AF.Sigmoid)

                # n = tanh(xn + r * (hn + bhh_n)); pmm_n holds hn+bhh_n
                tn2 = wpool.tile([128, 256], f16, tag="tn2")
                nc.vector.tensor_mul(tn2[:], pmm_n[:], rz[:, 0:256])
                sn = wpool.tile([128, 256], f16, tag="sn")
                nc.vector.tensor_add(sn[:], tn2[:], xw_n[:])
                n_t = wpool.tile([128, 256], f16, tag="n_t")
                nc.scalar.activation(n_t[:], sn[:], AF.Tanh)
                nc.scalar.activation(rz[:, 256:512], pmm_z[:], AF.Sigmoid)

                # e = z'*(n-h) (z' mask-frozen); h updates by += e in both
                # layouts. Tail is split into column halves so hT cols 0:128
                # (even k-tiles) are ready before the second half finishes.
                d_t = wpool.tile([128, 256], f16, tag="d_t")
                nc.vector.tensor_sub(d_t[:], n_t[:], h_str[:])
                e_t = wpool.tile([128, 256], f16, tag="e_t")
                tp = psum.tile([128, 256], f16, tag="tp", bufs=1)
                hT_new = wpool.tile([128, 256], f16, tag="hT", name="hT_new")
                for c in range(2):
                    cs = slice(128 * c, 128 * (c + 1))
                    nc.vector.tensor_mul(e_t[:, cs], d_t[:, cs],
                                         rz[:, 256 + 128 * c:384 + 128 * c])
                    nc.tensor.transpose(tp[:, cs], e_t[:, cs], ident_sb[:])
                for c in range(2):
                    cs = slice(128 * c, 128 * (c + 1))
                    nc.vector.tensor_add(hT_new[:, cs], tp[:, cs], hT[:, cs])
                h_new = wpool.tile([128, 256], f16, tag="h_str", name="h_new")
                nc.vector.tensor_add(h_new[:], e_t[:], h_str[:])

                h_str, hT = h_new, hT_new
                cur = nxt

            # ---- head ----------------------------------------------------
            ph = psum.tile([128, 512], f32, tag="p1", name="ph")
            for k in range(KT):
                coff = 128 * (k % 2) + 32 * (k // 2)
                nc.tensor.matmul(
                    ph[0:BL, 0:64],
                    hT[:, coff:coff + BL],
                    p1T_sb[:, 64 * k:64 * (k + 1)],
                    start=(k == 0), stop=(k == KT - 1),
                )
            t1s = wpool.tile([128, 64], f16, tag="t1s")
            nc.vector.tensor_add(t1s[0:BL, :], ph[0:BL, 0:64], p1b_sb[0:BL, :])
            t1 = wpool.tile([128, 64], f16, tag="t1")
            nc.scalar.activation(t1[0:BL, :], t1s[0:BL, :], AF.Tanh)

            pt1 = psum.tile([128, 256], f16, tag="tp", name="pt1", bufs=1)
            nc.tensor.transpose(pt1[0:64, 0:BL], t1[0:BL, 0:64], ident_sb[0:BL, 0:BL])
            t1T = wpool.tile([64, BL], f16, tag="t1T")
            nc.vector.tensor_copy(t1T[:], pt1[0:64, 0:BL])

            pl = psum.tile([128, 512], f32, tag="p1", name="pl")
            nc.tensor.matmul(pl[0:BL, 0:2], t1T[:], p2T_sb[:], start=True, stop=True)
            lg = wpool.tile([128, 2], f32, tag="lg")
            nc.vector.tensor_add(lg[0:BL, :], pl[0:BL, 0:2], p2b_sb[0:BL, :])
            nc.sync.dma_start(d_logits[:], lg[0:BL, :])

            if debug:
                nc.sync.dma_start(d_dbg_h[:], h_str[:])
                for g in ("src", "tgt"):
                    dbg = evpool.tile([128, 3072], f16, tag="dbgxw")
                    for mi in range(n_mstrip):
                        m0 = 128 * mi
                        msz = min(128, TB - m0)
                        nc.sync.dma_start(dbg[0:msz, :], xw_dram[g][m0:m0 + msz, :])
                        nc.sync.dma_start(d_dbg_xw[g][m0:m0 + msz, :], dbg[0:msz, :])

    nc.compile()
    return nc


# ----------------------------------------------------------------------------
# entry point
# ----------------------------------------------------------------------------

@functools.lru_cache(maxsize=2)
def _cached_program(n_steps, debug):
    return build_program(n_steps, debug)


def run(inputs, n_steps=T, debug=False, trace=False):
    inputs = {k: np.asarray(v) for k, v in inputs.items()}
    nc = _cached_program(n_steps, debug)
    shared = _prep_shared(inputs, n_steps)
    emb16 = np.asarray(inputs["emb"]).astype(np.float16)
    in_maps = []
    for c in range(NCORES):
        m = dict(shared)
        m.update(_prep_core(inputs, emb16, c, n_steps))
        in_maps.append(m)
    res = run_bass_kernel_spmd(nc, in_maps, list(range(NCORES)), trace=trace)
    logits = np.concatenate([res.results[c]["logits"] for c in range(NCORES)], axis=0)
    return logits, res


def kernel(**inputs) -> np.ndarray:
    logits, _ = run(inputs)
    return logits.astype(np.float32)



# revision 50
# speedup vs baseline: 1.1548x; 1.1548x over previous
"""Trainium2 Bass kernel for nn_Disc_53515292508892 (ragged_sequence).

Computes: src-GRU (H=1024) over ragged [128,64] token batch -> final hidden,
tgt-GRU seeded with it, then a 2-layer head -> logits [64, 2].
(The reference's ref-encoder outputs are computed then deleted -- dead code --
so they are skipped.)

Sharding: data-parallel over batch, B=64 -> 8 sequences per NeuronCore,
GRU weights replicated, no inter-core communication.

Per-core layout (fp16 compute, fp32 PSUM):
  - hidden state kept twice:
      h_str  [128, 256] : partition 32j+b (j = col-group, b = batch lane),
                          free = hidden unit within group (256 each)
      hT_buf [128, 256] : transposed (hidden-on-partition) = matmul lhsT
  - Whh reordered so col-group j holds (r_j | (1-z)_j | n_j) gate columns;
    recurrent matmul runs 4-way col-tiled via tile_position=(0,32j), so the
    four 768-column weight streams flow through the PE concurrently.
  - xW = x @ Wih.T (+ folded biases) precomputed on device into DRAM rows
    [(t*8+b), 3072], streamed back 48KB/step, double-buffered.

Perf ledger (HW exec, NTFF-profiled; this file ships the best config):
  2.758 ms  first correct version (single shared PSUM accumulator)
  2.10-2.14 ms  SHIPPED: split PSUM rz/n accumulators (sigma waits only on
                the 32 rz-matmuls; also keeps the PE gap under control)
  3.043 ms  REJECTED two-chain batch interleave -- the weight stream is
            batch-independent, so splitting the batch doubles PE work
  2.44-2.45 ms  REJECTED h-update refactor (p=m*z', h=p*n+(1-p)*h) +
                keep-warm dummy transposes + phase-1 dripping
  2.353 ms  REJECTED sigma r/z-half split + ACT/DVE parallel copies
  2.348 ms  REJECTED evens-first k-order + parallel copies alone
  (compile fail)  manual then_inc/_wait_ge to delay n-matmuls into the
                  sigma window: walrus setupSyncUpdate rejects an extra
                  sem update on a Tile-managed DVE op
Lesson: ops here are [128, 256-512] -- fixed per-op + semaphore-hop costs
(~150-250ns) dominate, so adding/splitting ops to shorten chain latency
loses; only wholesale work removal wins. Remaining known headroom
(~0.5 ms): HAM re-throttles the PE to 1.2 GHz during each step's ~3 us
gate window; needs dense real PE work in-window (half-hidden-chunk
pipeline) without net-new chain ops.
"""

import sys
import functools

sys.path.insert(0, "/opt/trn_rl_repo")

import numpy as np
import concourse.mybir as mybir
from concourse import bacc, tile
from concourse.bass_utils import run_bass_kernel_spmd

f16 = mybir.dt.float16
f32 = mybir.dt.float32
AO = mybir.AluOpType
AF = mybir.ActivationFunctionType

V, D, H = 32000, 512, 1024
T = 128          # steps per GRU (T_SRC = T_TGT = 128)
BL = 8           # batch per core
NCORES = 8
NG = 4           # col-tile groups
GW = 768         # gate columns per group (256 r | 256 z' | 256 n)
KT = H // 128    # 8 k-tiles over hidden
KD = D // 128    # 4 k-tiles over embedding dim


# ----------------------------------------------------------------------------
# host-side weight/layout prep
# ----------------------------------------------------------------------------

def _gate_perm():
    """perm[c] = original Whh/Wih row for reordered gate column c.
    Gate-major layout [r 1024 | n 1024 | z' 1024]; within a gate, unit
    256j+u belongs to quadrant group j. One DMA each for the per-step r/z
    blocks; sign = -1 for z' columns (z' = 1-z = sigmoid(-pre_z))."""
    c = np.arange(3072)
    gate = c // 1024
    unit = c % 1024
    row = np.where(gate == 0, unit,
                   np.where(gate == 1, 2048 + unit, 1024 + unit))
    sign = np.where(gate == 2, -1.0, 1.0).astype(np.float32)
    is_n = gate == 1
    return row, sign, is_n


def _prep_shared(inputs, n_steps):
    """Core-independent tensors (weights, biases, identity)."""
    row, sign, is_n = _gate_perm()
    out = {}
    for g, wih, whh, bih, bhh in (
        ("src", inputs["src_Wih"], inputs["src_Whh"], inputs["src_bih"], inputs["src_bhh"]),
        ("tgt", inputs["tgt_Wih"], inputs["tgt_Whh"], inputs["tgt_bih"], inputs["tgt_bhh"]),
    ):
        whh_a = (whh[row] * sign[:, None]).T.astype(np.float16)          # [1024, 3072]
        out[f"whh_{g}"] = np.ascontiguousarray(whh_a.reshape(KT, 128, 3072))
        wih_a = (wih[row] * sign[:, None]).T.astype(np.float16)          # [512, 3072]
        out[f"wih_{g}"] = np.ascontiguousarray(wih_a.reshape(KD, 128, 3072))
        bias_vec = sign * bih[row] + sign * np.where(is_n, 0.0, bhh[row])
        out[f"bias_{g}"] = np.broadcast_to(
            bias_vec.astype(np.float16), (128, 3072)).copy()
        # bhh for n-gate on one partition: col 256j+u = bhh[2048+256j+u];
        # streamed into PSUM via a k=1 ones-matmul preload
        bhnr = np.zeros((1, 1024), np.float16)
        for j in range(NG):
            bhnr[0, 256 * j:256 * (j + 1)] = bhh[2048 + 256 * j:2048 + 256 * (j + 1)].astype(np.float16)
        out[f"bhnr_{g}"] = bhnr
    p1 = inputs["p1_W"].T.reshape(KT, 128, 64).transpose(1, 0, 2).reshape(128, KT * 64)
    out["p1T"] = p1.astype(np.float16)
    out["p1b"] = np.broadcast_to(inputs["p1_b"].astype(np.float16), (128, 64)).copy()
    out["p2T"] = inputs["p2_W"].T.astype(np.float16)                      # [64, 2]
    out["p2b"] = np.broadcast_to(inputs["p2_b"].astype(np.float32), (128, 2)).copy()
    out["ident"] = np.eye(128, dtype=np.float16)
    out["ones1"] = np.ones((1, BL), np.float16)
    return out


def _prep_core(inputs, emb16, core, n_steps):
    """Per-core tensors: gathered/transposed token embeddings and masks."""
    sl = slice(BL * core, BL * (core + 1))
    out = {}
    for g, ids_key, len_key in (("src", "src", "src_lengths"),
                                ("tgt", "tgt", "tgt_lengths")):
        ids = np.asarray(inputs[ids_key])[:n_steps, sl]                   # [T, 8]
        x = emb16[ids]                                                    # [T, 8, 512]
        out[f"xT_{g}"] = np.ascontiguousarray(
            x.transpose(2, 0, 1).reshape(KD, 128, n_steps * BL))
    # per-partition z'-gate mask bias for phase 1: xw row r = t*BL + b of
    # m-strip mi sits on partition p = r - 128*mi; -60 on z' cols where
    # t >= len freezes h (sigmoid(-60+pre) ~ 0)
    n_mstrip = (n_steps * BL + 127) // 128
    mb = np.zeros((128, 2 * n_mstrip), np.float32)
    for gi, len_key in enumerate(("src_lengths", "tgt_lengths")):
        ln = np.asarray(inputs[len_key])[sl]                              # [8]
        r = np.arange(n_steps * BL)
        masked = (r // BL) >= ln[r % BL]                                  # [T*BL]
        mb[:, gi * n_mstrip:(gi + 1) * n_mstrip] = np.where(
            masked, -60.0, 0.0).reshape(n_mstrip, 128).T
    out["maskbias"] = mb
    return out


# ----------------------------------------------------------------------------
# device program
# ----------------------------------------------------------------------------

def build_program(n_steps=T, debug=False):
    nc = bacc.Bacc("TRN2", target_bir_lowering=False, debug=False,
                   num_devices=NCORES)
    TB = n_steps * BL

    dp = nc.declare_dram_parameter
    d_xT = {g: dp(f"xT_{g}", [KD, 128, TB], f16, isOutput=False) for g in ("src", "tgt")}
    d_whh = {g: dp(f"whh_{g}", [KT, 128, 3072], f16, isOutput=False) for g in ("src", "tgt")}
    d_wih = {g: dp(f"wih_{g}", [KD, 128, 3072], f16, isOutput=False) for g in ("src", "tgt")}
    d_bias = {g: dp(f"bias_{g}", [128, 3072], f16, isOutput=False) for g in ("src", "tgt")}
    d_bhnr = {g: dp(f"bhnr_{g}", [1, 1024], f16, isOutput=False) for g in ("src", "tgt")}
    n_mstrip_d = (TB + 127) // 128
    d_maskbias = dp("maskbias", [128, 2 * n_mstrip_d], f32, isOutput=False)
    d_ones1 = dp("ones1", [1, BL], f16, isOutput=False)
    d_p1T = dp("p1T", [128, KT * 64], f16, isOutput=False)
    d_p1b = dp("p1b", [128, 64], f16, isOutput=False)
    d_p2T = dp("p2T", [64, 2], f16, isOutput=False)
    d_p2b = dp("p2b", [128, 2], f32, isOutput=False)
    d_ident = dp("ident", [128, 128], f16, isOutput=False)
    d_logits = dp("logits", [BL, 2], f32, isOutput=True)
    if debug:
        d_dbg_h = dp("dbg_h", [128, 256], f16, isOutput=True)
        d_dbg_xw = {g: dp(f"dbg_xw_{g}", [TB, 3072], f16, isOutput=True)
                    for g in ("src", "tgt")}

    with tile.TileContext(nc) as tc:
        with tc.tile_pool(name="const", bufs=1) as cpool, \
             tc.tile_pool(name="work", bufs=2) as wpool, \
             tc.tile_pool(name="xwload", bufs=4) as xwpool, \
             tc.tile_pool(name="p1ev", bufs=4) as evpool, \
             tc.tile_pool(name="psum", bufs=2, space="PSUM") as psum, \
             tc.tile_pool(name="dram", bufs=1, space="DRAM") as dram:

            # ---- resident constants -------------------------------------
            whh_sb, xT_sb, bias_sb, bhhn_sb = {}, {}, {}, {}
            for g in ("src", "tgt"):
                whh_sb[g] = cpool.tile([128, KT * 3072], f16, tag=f"whh_{g}", name=f"whh_{g}")
                for k in range(KT):
                    nc.sync.dma_start(whh_sb[g][:, 3072 * k:3072 * (k + 1)], d_whh[g][k])
                xT_sb[g] = cpool.tile([128, KD * TB], f16, tag=f"xT_{g}", name=f"xT_{g}")
                for k in range(KD):
                    nc.sync.dma_start(xT_sb[g][:, TB * k:TB * (k + 1)], d_xT[g][k])
                bias_sb[g] = cpool.tile([128, 3072], f16, tag=f"bias_{g}", name=f"biassb_{g}")
                nc.sync.dma_start(bias_sb[g][:], d_bias[g][:])
                bhhn_sb[g] = cpool.tile([1, 1024], f16, tag=f"bhnr_{g}", name=f"bhnrsb_{g}")
                nc.sync.dma_start(bhhn_sb[g][:], d_bhnr[g][:])
            mb_sb = cpool.tile([128, 2 * n_mstrip_d], f32, tag="maskbias")
            nc.sync.dma_start(mb_sb[:], d_maskbias[:])
            ident_sb = cpool.tile([128, 128], f16, tag="ident")
            nc.sync.dma_start(ident_sb[:], d_ident[:])
            ones1_sb = cpool.tile([1, BL], f16, tag="ones1")
            nc.sync.dma_start(ones1_sb[:], d_ones1[:])
            p1T_sb = cpool.tile([128, KT * 64], f16, tag="p1T")
            nc.sync.dma_start(p1T_sb[:], d_p1T[:])
            p1b_sb = cpool.tile([128, 64], f16, tag="p1b")
            nc.sync.dma_start(p1b_sb[:], d_p1b[:])
            p2T_sb = cpool.tile([64, 2], f16, tag="p2T")
            nc.sync.dma_start(p2T_sb[:], d_p2T[:])
            p2b_sb = cpool.tile([128, 2], f32, tag="p2b")
            nc.sync.dma_start(p2b_sb[:], d_p2b[:])

            # ---- phase 1 (as a generator of work units so tgt can be
            # interleaved into the src recurrence to fill PE gaps) ---------
            xw_dram = {}
            for g in ("src", "tgt"):
                xw_dram[g] = dram.tile([TB, 3072], f16, tag=f"xw_{g}", name=f"xwdram_{g}")
            n_mstrip = (TB + 127) // 128

            def phase1_units(g):
                for ch in range(6):
                    wihs = []
                    for kd in range(KD):
                        wt = evpool.tile([128, 512], f16, tag="wih_s", name="wih_s",
                                         bufs=8)
                        nc.sync.dma_start(
                            wt[:], d_wih[g][kd, :, 512 * ch:512 * (ch + 1)])
                        wihs.append(wt)
                    for mi in range(n_mstrip):
                        m0 = 128 * mi
                        msz = min(128, TB - m0)
                        ps = psum.tile([128, 512], f32, tag="p1", name="p1ps")
                        for kd in range(KD):
                            nc.tensor.matmul(
                                ps[0:msz, :],
                                xT_sb[g][:, TB * kd + m0: TB * kd + m0 + msz],
                                wihs[kd][:],
                                start=(kd == 0), stop=(kd == KD - 1),
                            )
                        ev = evpool.tile([128, 512], f16, tag="ev", name="ev")
                        if ch < 4:
                            nc.vector.tensor_add(
                                ev[0:msz, :], ps[0:msz, :],
                                bias_sb[g][0:msz, 512 * ch:512 * (ch + 1)])
                        else:
                            # chunks 4-5 are all z' columns: also add the
                            # per-row -60 freeze bias (masks h past length)
                            gi = 0 if g == "src" else 1
                            nc.vector.scalar_tensor_tensor(
                                ev[0:msz, :], ps[0:msz, :],
                                mb_sb[0:msz, gi * n_mstrip_d + mi:
                                      gi * n_mstrip_d + mi + 1],
                                bias_sb[g][0:msz, 512 * ch:512 * (ch + 1)],
                                AO.add, AO.add)
                        nc.sync.dma_start(
                            xw_dram[g][m0:m0 + msz, 512 * ch:512 * (ch + 1)],
                            ev[0:msz, :])
                        yield

            for g in ("src", "tgt"):
                for _ in phase1_units(g):
                    pass

            # ---- recurrence ---------------------------------------------
            h_str = wpool.tile([128, 256], f16, tag="h_str")
            hT = wpool.tile([128, 256], f16, tag="hT")
            nc.vector.memset(h_str[:], 0.0)
            nc.vector.memset(hT[:], 0.0)


            def stage_in(step):
                """DMA xw + psum accumulators + PE preloads for `step`.
                Called one step ahead so the tiny preload matmuls (8/1-row
                stationaries: high busy, ~6% array power) land in the PE's
                post-stream idle window -- real keep-warm work for the HAM
                clock gate -- and the xw adds never touch the gate chain."""
                g = "src" if step < n_steps else "tgt"
                r0 = (step % n_steps) * BL
                # xw r/z blocks at base partition 0 (stationary base partition
                # must equal the PE tile row); xn in strip layout for sn
                xw_rb = xwpool.tile([BL, 1024], f16, tag="xw_rb", name="xw_rb")
                xw_zb = xwpool.tile([BL, 1024], f16, tag="xw_zb", name="xw_zb")
                xw_n = xwpool.tile([128, 256], f16, tag="xw_n", name="xw_n")
                nc.sync.dma_start(xw_rb[:], xw_dram[g][r0:r0 + BL, 0:1024])
                nc.sync.dma_start(xw_zb[:], xw_dram[g][r0:r0 + BL, 2048:3072])
                for j in range(NG):
                    nc.sync.dma_start(
                        xw_n[32 * j:32 * j + BL, :],
                        xw_dram[g][r0:r0 + BL, 1024 + 256 * j:1024 + 256 * (j + 1)])
                pmm_r = psum.tile([128, 256], f32, tag="mm_r", name="pmm_r",
                                  bufs=1)
                pmm_n = psum.tile([128, 256], f32, tag="mm_n", name="pmm_n",
                                  bufs=1)
                pmm_z = psum.tile([128, 256], f32, tag="mm_z", name="pmm_z",
                                  bufs=1)
                for j in range(NG):
                    nc.tensor.matmul(
                        pmm_r[32 * j:32 * j + BL, :],
                        ident_sb[0:BL, 0:BL],
                        xw_rb[:, 256 * j:256 * (j + 1)],
                        start=True, stop=False, tile_position=(0, 32 * j))
                for j in range(NG):
                    nc.tensor.matmul(
                        pmm_n[32 * j:32 * j + BL, :],
                        ones1_sb[0:1, :],
                        bhhn_sb[g][0:1, 256 * j:256 * (j + 1)],
                        start=True, stop=False, tile_position=(0, 32 * j))
                for j in range(NG):
                    nc.tensor.matmul(
                        pmm_z[32 * j:32 * j + BL, :],
                        ident_sb[0:BL, 0:BL],
                        xw_zb[:, 256 * j:256 * (j + 1)],
                        start=True, stop=False, tile_position=(0, 32 * j))
                return g, pmm_r, pmm_n, pmm_z, xw_n

            cur = stage_in(0)
            for step in range(2 * n_steps):
                g, pmm_r, pmm_n, pmm_z, xw_n = cur

                # three 256-col weight streams r -> n -> z: sig_r fires after
                # the first, the gate chain overlaps the rest. Even k-tiles
                # first: they read hT cols 0:128, produced by the first half
                # of the previous step's split transpose/hT-update.
                for c0, dst in ((0, pmm_r), (1024, pmm_n), (2048, pmm_z)):
                    for ki, k in enumerate((0, 2, 4, 6, 1, 3, 5, 7)):
                        coff = 128 * (k % 2) + 32 * (k // 2)
                        lhsT = hT[:, coff:coff + BL]
                        for j in range(NG):
                            nc.tensor.matmul(
                                dst[32 * j:32 * j + BL, :],
                                lhsT,
                                whh_sb[g][:, 3072 * k + c0 + 256 * j:
                                        3072 * k + c0 + 256 * (j + 1)],
                                start=False, stop=(ki == KT - 1),
                                tile_position=(0, 32 * j),
                            )

                # stage the next step now: its DMAs + PE preloads fill the
                # idle window between the z-stream and the transposes
                nxt = stage_in(step + 1) if step + 1 < 2 * n_steps else None

                # gates (strip view [128, *]; only partitions 32j+b<8 valid);
                # sigmoids read PSUM directly
                rz = wpool.tile([128, 512], f16, tag="rz")
                nc.scalar.activation(rz[:, 0:256], pmm_r[:], AF.Sigmoid)

                # n = tanh(xn + r * (hn + bhh_n)); pmm_n holds hn+bhh_n
                tn2 = wpool.tile([128, 256], f16, tag="tn2")
                nc.vector.tensor_mul(tn2[:], pmm_n[:], rz[:, 0:256])
                sn = wpool.tile([128, 256], f16, tag="sn")
                nc.vector.tensor_add(sn[:], tn2[:], xw_n[:])
                n_t = wpool.tile([128, 256], f16, tag="n_t")
                nc.scalar.activation(n_t[:], sn[:], AF.Tanh)
                nc.scalar.activation(rz[:, 256:512], pmm_z[:], AF.Sigmoid)

                # e = z'*(n-h) (z' mask-frozen); h updates by += e in both
                # layouts. Tail is split into column halves so hT cols 0:128
                # (even k-tiles) are ready before the second half finishes.
                d_t = wpool.tile([128, 256], f16, tag="d_t")
                nc.vector.tensor_sub(d_t[:], n_t[:], h_str[:])
                e_t = wpool.tile([128, 256], f16, tag="e_t")
                tp = psum.tile([128, 256], f16, tag="tp", bufs=1)
                hT_new = wpool.tile([128, 256], f16, tag="hT", name="hT_new")
                for c in range(2):
                    cs = slice(128 * c, 128 * (c + 1))
                    nc.vector.tensor_mul(e_t[:, cs], d_t[:, cs],
                                         rz[:, 256 + 128 * c:384 + 128 * c])
                    nc.tensor.transpose(tp[:, cs], e_t[:, cs], ident_sb[:])
                for c in range(2):
                    cs = slice(128 * c, 128 * (c + 1))
                    nc.vector.tensor_add(hT_new[:, cs], tp[:, cs], hT[:, cs])
                h_new = wpool.tile([128, 256], f16, tag="h_str", name="h_new")
                nc.gpsimd.tensor_add(h_new[:], e_t[:], h_str[:])

                h_str, hT = h_new, hT_new
                cur = nxt

            # ---- head ----------------------------------------------------
            ph = psum.tile([128, 512], f32, tag="p1", name="ph")
            for k in range(KT):
                coff = 128 * (k % 2) + 32 * (k // 2)
                nc.tensor.matmul(
                    ph[0:BL, 0:64],
                    hT[:, coff:coff + BL],
                    p1T_sb[:, 64 * k:64 * (k + 1)],
                    start=(k == 0), stop=(k == KT - 1),
                )
            t1s = wpool.tile([128, 64], f16, tag="t1s")
            nc.vector.tensor_add(t1s[0:BL, :], ph[0:BL, 0:64], p1b_sb[0:BL, :])
            t1 = wpool.tile([128, 64], f16, tag="t1")
            nc.scalar.activation(t1[0:BL, :], t1s[0:BL, :], AF.Tanh)

            pt1 = psum.tile([128, 256], f16, tag="tp", name="pt1", bufs=1)
            nc.tensor.transpose(pt1[0:64, 0:BL], t1[0:BL, 0:64], ident_sb[0:BL, 0:BL])
            t1T = wpool.tile([64, BL], f16, tag="t1T")
            nc.vector.tensor_copy(t1T[:], pt1[0:64, 0:BL])

            pl = psum.tile([128, 512], f32, tag="p1", name="pl")
            nc.tensor.matmul(pl[0:BL, 0:2], t1T[:], p2T_sb[:], start=True, stop=True)
            lg = wpool.tile([128, 2], f32, tag="lg")
            nc.vector.tensor_add(lg[0:BL, :], pl[0:BL, 0:2], p2b_sb[0:BL, :])
            nc.sync.dma_start(d_logits[:], lg[0:BL, :])

            if debug:
                nc.sync.dma_start(d_dbg_h[:], h_str[:])
                for g in ("src", "tgt"):
                    dbg = evpool.tile([128, 3072], f16, tag="dbgxw")
                    for mi in range(n_mstrip):
                        m0 = 128 * mi
                        msz = min(128, TB - m0)
                        nc.sync.dma_start(dbg[0:msz, :], xw_dram[g][m0:m0 + msz, :])
                        nc.sync.dma_start(d_dbg_xw[g][m0:m0 + msz, :], dbg[0:msz, :])

    nc.compile()
    return nc


# ----------------------------------------------------------------------------
# entry point
# ----------------------------------------------------------------------------

@functools.lru_cache(maxsize=2)
def _cached_program(n_steps, debug):
    return build_program(n_steps, debug)


def run(inputs, n_steps=T, debug=False, trace=False):
    inputs = {k: np.asarray(v) for k, v in inputs.items()}
    nc = _cached_program(n_steps, debug)
    shared = _prep_shared(inputs, n_steps)
    emb16 = np.asarray(inputs["emb"]).astype(np.float16)
    in_maps = []
    for c in range(NCORES):
        m = dict(shared)
        m.update(_prep_core(inputs, emb16, c, n_steps))
        in_maps.append(m)
    res = run_bass_kernel_spmd(nc, in_maps, list(range(NCORES)), trace=trace)
    logits = np.concatenate([res.results[c]["logits"] for c in range(NCORES)], axis=0)
    return logits, res


def kernel(**inputs) -> np.ndarray:
    logits, _ = run(inputs)
    return logits.astype(np.float32)



# revision 51
# speedup vs baseline: 1.1622x; 1.0064x over previous
"""Trainium2 Bass kernel for nn_Disc_53515292508892 (ragged_sequence).

Computes: src-GRU (H=1024) over ragged [128,64] token batch -> final hidden,
tgt-GRU seeded with it, then a 2-layer head -> logits [64, 2].
(The reference's ref-encoder outputs are computed then deleted -- dead code --
so they are skipped.)

Sharding: data-parallel over batch, B=64 -> 8 sequences per NeuronCore,
GRU weights replicated, no inter-core communication.

Per-core layout (fp16 compute, fp32 PSUM):
  - hidden state kept twice:
      h_str  [128, 256] : partition 32j+b (j = col-group, b = batch lane),
                          free = hidden unit within group (256 each)
      hT_buf [128, 256] : transposed (hidden-on-partition) = matmul lhsT
  - Whh/Wih/xw reordered GATE-MAJOR [r 1024 | n 1024 | z' 1024] (z' flipped:
    z' = 1-z = sigmoid(-pre_z), with a -60 bias on rows past each sequence
    length so masked steps freeze h exactly -- no mask op in the loop).
  - per step, three 256-col weight streams r -> n -> z, each 8 k-tiles x 4
    quadrant matmuls (tile_position=(0,32j)); sig_r fires after the first
    stream, the n/z streams run under the gate chain.
  - PSUM accumulators are PRELOADED (start=False hT matmuls accumulate on
    top): pmm_r/pmm_z via tiny k=8 identity matmuls from the DMA'd xw
    blocks, pmm_n via a k=1 ones matmul of bhh_n -- so no xw/bias adds
    remain in the serial chain, and the preloads double as low-power
    keep-warm PE work in the post-stream window (HAM clock gate).
  - stage_in(step+1) is emitted mid-step (software pipelining) so its DMAs
    and preloads land in the PE idle window.
  - chain: sig_r(ACT, reads PSUM) -> tn2=r*pmm_n (DVE) -> sn=tn2+xn ->
    tanh(ACT) -> sub -> e=z'*(n-h) in column halves -> transpose-of-e (PE)
    -> hT += e^T in halves (DVE; even k-tiles of the next step's streams
    start after the first half) -> h += e on GpSimd (off critical path).
  - xW = x @ Wih.T (+ biases + mask) precomputed on device into DRAM rows
    [(t*8+b), 3072], streamed back 48KB/step, double-buffered.

Perf ledger (HW exec, NTFF-profiled; this file ships the best config):
  2.09-2.35 ms  inherited baseline (rz/n split accumulators)
  2.50 ms  REJECTED PSUM preload via wide PE identity matmuls w/ group-major
           layout (tile-row rule: stationary base partition == tile row;
           and the fatter PE bursts ran the whole stream at K=4/8 cold)
  2.06 ms  ACT-copy preloads (chain shortened, streams still cold)
  1.74 ms  3x 256-col per-gate streams r->n->z + bufs=1 psum: sig_r after
           1/3 of the stream; z overlaps the chain
  2.21-2.30 ms  REJECTED [r|n] 512-col combined stream; REJECTED 12-DMA
           group-major xw staging (preloads dribbled, blocked transposes)
  1.65 ms  gate-major layout (1-DMA xw blocks) + stage-ahead PE preloads:
           streams run warm (109ns/k-tile)
  1.63-1.69 ms  SHIPPED: + tail split in halves (even k-tiles early) +
           h_new on GpSimd (suppresses the slow phase-lock mode)
Known structure of the remaining time (avg ~6.5us/step x 256):
  streams ~3.3us (avg over the fixed 17.07us HAM duty cycle: 3.4us at
  K=4/8 cold, 13.7us warm -- a ~20% tax we cannot remove), exposed gate
  chain ~2.2us (tn2/sn/tanh/sub/e + 2 PE-sem wakeups at 140-650ns), tail
  ~1us. Next structural win would be half-hidden-unit pipelining (stream
  half-B under chain-A), est. -0.5..-0.9us/step but +6 sem hops of risk.
HW exec is BIMODAL run-to-run (~1.65 vs ~1.94 ms): the free-running
throttle cycle phase-locks against the step period. Min-of-N when
benchmarking.
"""

import sys
import functools

sys.path.insert(0, "/opt/trn_rl_repo")

import numpy as np
import concourse.mybir as mybir
from concourse import bacc, tile
from concourse.bass_utils import run_bass_kernel_spmd

f16 = mybir.dt.float16
f32 = mybir.dt.float32
AO = mybir.AluOpType
AF = mybir.ActivationFunctionType

V, D, H = 32000, 512, 1024
T = 128          # steps per GRU (T_SRC = T_TGT = 128)
BL = 8           # batch per core
NCORES = 8
NG = 4           # col-tile groups
GW = 768         # gate columns per group (256 r | 256 z' | 256 n)
KT = H // 128    # 8 k-tiles over hidden
KD = D // 128    # 4 k-tiles over embedding dim


# ----------------------------------------------------------------------------
# host-side weight/layout prep
# ----------------------------------------------------------------------------

def _gate_perm():
    """perm[c] = original Whh/Wih row for reordered gate column c.
    Gate-major layout [r 1024 | n 1024 | z' 1024]; within a gate, unit
    256j+u belongs to quadrant group j. One DMA each for the per-step r/z
    blocks; sign = -1 for z' columns (z' = 1-z = sigmoid(-pre_z))."""
    c = np.arange(3072)
    gate = c // 1024
    unit = c % 1024
    row = np.where(gate == 0, unit,
                   np.where(gate == 1, 2048 + unit, 1024 + unit))
    sign = np.where(gate == 2, -1.0, 1.0).astype(np.float32)
    is_n = gate == 1
    return row, sign, is_n


def _prep_shared(inputs, n_steps):
    """Core-independent tensors (weights, biases, identity)."""
    row, sign, is_n = _gate_perm()
    out = {}
    for g, wih, whh, bih, bhh in (
        ("src", inputs["src_Wih"], inputs["src_Whh"], inputs["src_bih"], inputs["src_bhh"]),
        ("tgt", inputs["tgt_Wih"], inputs["tgt_Whh"], inputs["tgt_bih"], inputs["tgt_bhh"]),
    ):
        whh_a = (whh[row] * sign[:, None]).T.astype(np.float16)          # [1024, 3072]
        out[f"whh_{g}"] = np.ascontiguousarray(whh_a.reshape(KT, 128, 3072))
        wih_a = (wih[row] * sign[:, None]).T.astype(np.float16)          # [512, 3072]
        out[f"wih_{g}"] = np.ascontiguousarray(wih_a.reshape(KD, 128, 3072))
        bias_vec = sign * bih[row] + sign * np.where(is_n, 0.0, bhh[row])
        out[f"bias_{g}"] = np.broadcast_to(
            bias_vec.astype(np.float16), (128, 3072)).copy()
        # bhh for n-gate on one partition: col 256j+u = bhh[2048+256j+u];
        # streamed into PSUM via a k=1 ones-matmul preload
        bhnr = np.zeros((1, 1024), np.float16)
        for j in range(NG):
            bhnr[0, 256 * j:256 * (j + 1)] = bhh[2048 + 256 * j:2048 + 256 * (j + 1)].astype(np.float16)
        out[f"bhnr_{g}"] = bhnr
    p1 = inputs["p1_W"].T.reshape(KT, 128, 64).transpose(1, 0, 2).reshape(128, KT * 64)
    out["p1T"] = p1.astype(np.float16)
    out["p1b"] = np.broadcast_to(inputs["p1_b"].astype(np.float16), (128, 64)).copy()
    out["p2T"] = inputs["p2_W"].T.astype(np.float16)                      # [64, 2]
    out["p2b"] = np.broadcast_to(inputs["p2_b"].astype(np.float32), (128, 2)).copy()
    out["ident"] = np.eye(128, dtype=np.float16)
    out["ones1"] = np.ones((1, BL), np.float16)
    return out


def _prep_core(inputs, emb16, core, n_steps):
    """Per-core tensors: gathered/transposed token embeddings and masks."""
    sl = slice(BL * core, BL * (core + 1))
    out = {}
    for g, ids_key, len_key in (("src", "src", "src_lengths"),
                                ("tgt", "tgt", "tgt_lengths")):
        ids = np.asarray(inputs[ids_key])[:n_steps, sl]                   # [T, 8]
        x = emb16[ids]                                                    # [T, 8, 512]
        out[f"xT_{g}"] = np.ascontiguousarray(
            x.transpose(2, 0, 1).reshape(KD, 128, n_steps * BL))
    # per-partition z'-gate mask bias for phase 1: xw row r = t*BL + b of
    # m-strip mi sits on partition p = r - 128*mi; -60 on z' cols where
    # t >= len freezes h (sigmoid(-60+pre) ~ 0)
    n_mstrip = (n_steps * BL + 127) // 128
    mb = np.zeros((128, 2 * n_mstrip), np.float32)
    for gi, len_key in enumerate(("src_lengths", "tgt_lengths")):
        ln = np.asarray(inputs[len_key])[sl]                              # [8]
        r = np.arange(n_steps * BL)
        masked = (r // BL) >= ln[r % BL]                                  # [T*BL]
        mb[:, gi * n_mstrip:(gi + 1) * n_mstrip] = np.where(
            masked, -60.0, 0.0).reshape(n_mstrip, 128).T
    out["maskbias"] = mb
    return out


# ----------------------------------------------------------------------------
# device program
# ----------------------------------------------------------------------------

def build_program(n_steps=T, debug=False):
    nc = bacc.Bacc("TRN2", target_bir_lowering=False, debug=False,
                   num_devices=NCORES)
    TB = n_steps * BL

    dp = nc.declare_dram_parameter
    d_xT = {g: dp(f"xT_{g}", [KD, 128, TB], f16, isOutput=False) for g in ("src", "tgt")}
    d_whh = {g: dp(f"whh_{g}", [KT, 128, 3072], f16, isOutput=False) for g in ("src", "tgt")}
    d_wih = {g: dp(f"wih_{g}", [KD, 128, 3072], f16, isOutput=False) for g in ("src", "tgt")}
    d_bias = {g: dp(f"bias_{g}", [128, 3072], f16, isOutput=False) for g in ("src", "tgt")}
    d_bhnr = {g: dp(f"bhnr_{g}", [1, 1024], f16, isOutput=False) for g in ("src", "tgt")}
    n_mstrip_d = (TB + 127) // 128
    d_maskbias = dp("maskbias", [128, 2 * n_mstrip_d], f32, isOutput=False)
    d_ones1 = dp("ones1", [1, BL], f16, isOutput=False)
    d_p1T = dp("p1T", [128, KT * 64], f16, isOutput=False)
    d_p1b = dp("p1b", [128, 64], f16, isOutput=False)
    d_p2T = dp("p2T", [64, 2], f16, isOutput=False)
    d_p2b = dp("p2b", [128, 2], f32, isOutput=False)
    d_ident = dp("ident", [128, 128], f16, isOutput=False)
    d_logits = dp("logits", [BL, 2], f32, isOutput=True)
    if debug:
        d_dbg_h = dp("dbg_h", [128, 256], f16, isOutput=True)
        d_dbg_xw = {g: dp(f"dbg_xw_{g}", [TB, 3072], f16, isOutput=True)
                    for g in ("src", "tgt")}

    with tile.TileContext(nc) as tc:
        with tc.tile_pool(name="const", bufs=1) as cpool, \
             tc.tile_pool(name="work", bufs=2) as wpool, \
             tc.tile_pool(name="xwload", bufs=4) as xwpool, \
             tc.tile_pool(name="p1ev", bufs=4) as evpool, \
             tc.tile_pool(name="psum", bufs=2, space="PSUM") as psum, \
             tc.tile_pool(name="dram", bufs=1, space="DRAM") as dram:

            # ---- resident constants -------------------------------------
            whh_sb, xT_sb, bias_sb, bhhn_sb = {}, {}, {}, {}
            for g in ("src", "tgt"):
                whh_sb[g] = cpool.tile([128, KT * 3072], f16, tag=f"whh_{g}", name=f"whh_{g}")
                for k in range(KT):
                    nc.sync.dma_start(whh_sb[g][:, 3072 * k:3072 * (k + 1)], d_whh[g][k])
                xT_sb[g] = cpool.tile([128, KD * TB], f16, tag=f"xT_{g}", name=f"xT_{g}")
                for k in range(KD):
                    nc.sync.dma_start(xT_sb[g][:, TB * k:TB * (k + 1)], d_xT[g][k])
                bias_sb[g] = cpool.tile([128, 3072], f16, tag=f"bias_{g}", name=f"biassb_{g}")
                nc.sync.dma_start(bias_sb[g][:], d_bias[g][:])
                bhhn_sb[g] = cpool.tile([1, 1024], f16, tag=f"bhnr_{g}", name=f"bhnrsb_{g}")
                nc.sync.dma_start(bhhn_sb[g][:], d_bhnr[g][:])
            mb_sb = cpool.tile([128, 2 * n_mstrip_d], f32, tag="maskbias")
            nc.sync.dma_start(mb_sb[:], d_maskbias[:])
            ident_sb = cpool.tile([128, 128], f16, tag="ident")
            nc.sync.dma_start(ident_sb[:], d_ident[:])
            ones1_sb = cpool.tile([1, BL], f16, tag="ones1")
            nc.sync.dma_start(ones1_sb[:], d_ones1[:])
            p1T_sb = cpool.tile([128, KT * 64], f16, tag="p1T")
            nc.sync.dma_start(p1T_sb[:], d_p1T[:])
            p1b_sb = cpool.tile([128, 64], f16, tag="p1b")
            nc.sync.dma_start(p1b_sb[:], d_p1b[:])
            p2T_sb = cpool.tile([64, 2], f16, tag="p2T")
            nc.sync.dma_start(p2T_sb[:], d_p2T[:])
            p2b_sb = cpool.tile([128, 2], f32, tag="p2b")
            nc.sync.dma_start(p2b_sb[:], d_p2b[:])

            # ---- phase 1 (as a generator of work units so tgt can be
            # interleaved into the src recurrence to fill PE gaps) ---------
            xw_dram = {}
            for g in ("src", "tgt"):
                xw_dram[g] = dram.tile([TB, 3072], f16, tag=f"xw_{g}", name=f"xwdram_{g}")
            n_mstrip = (TB + 127) // 128

            def phase1_units(g):
                for ch in range(6):
                    wihs = []
                    for kd in range(KD):
                        wt = evpool.tile([128, 512], f16, tag="wih_s", name="wih_s",
                                         bufs=8)
                        nc.sync.dma_start(
                            wt[:], d_wih[g][kd, :, 512 * ch:512 * (ch + 1)])
                        wihs.append(wt)
                    for mi in range(n_mstrip):
                        m0 = 128 * mi
                        msz = min(128, TB - m0)
                        ps = psum.tile([128, 512], f32, tag="p1", name="p1ps")
                        for kd in range(KD):
                            nc.tensor.matmul(
                                ps[0:msz, :],
                                xT_sb[g][:, TB * kd + m0: TB * kd + m0 + msz],
                                wihs[kd][:],
                                start=(kd == 0), stop=(kd == KD - 1),
                            )
                        ev = evpool.tile([128, 512], f16, tag="ev", name="ev")
                        if ch < 4:
                            nc.vector.tensor_add(
                                ev[0:msz, :], ps[0:msz, :],
                                bias_sb[g][0:msz, 512 * ch:512 * (ch + 1)])
                        else:
                            # chunks 4-5 are all z' columns: also add the
                            # per-row -60 freeze bias (masks h past length)
                            gi = 0 if g == "src" else 1
                            nc.vector.scalar_tensor_tensor(
                                ev[0:msz, :], ps[0:msz, :],
                                mb_sb[0:msz, gi * n_mstrip_d + mi:
                                      gi * n_mstrip_d + mi + 1],
                                bias_sb[g][0:msz, 512 * ch:512 * (ch + 1)],
                                AO.add, AO.add)
                        nc.sync.dma_start(
                            xw_dram[g][m0:m0 + msz, 512 * ch:512 * (ch + 1)],
                            ev[0:msz, :])
                        yield

            for g in ("src", "tgt"):
                for _ in phase1_units(g):
                    pass

            # ---- recurrence ---------------------------------------------
            h_str = wpool.tile([128, 256], f16, tag="h_str")
            hT = wpool.tile([128, 256], f16, tag="hT")
            nc.vector.memset(h_str[:], 0.0)
            nc.vector.memset(hT[:], 0.0)


            def stage_in(step):
                """DMA xw + psum accumulators + PE preloads for `step`.
                Called one step ahead so the tiny preload matmuls (8/1-row
                stationaries: high busy, ~6% array power) land in the PE's
                post-stream idle window -- real keep-warm work for the HAM
                clock gate -- and the xw adds never touch the gate chain."""
                g = "src" if step < n_steps else "tgt"
                r0 = (step % n_steps) * BL
                # xw r/z blocks at base partition 0 (stationary base partition
                # must equal the PE tile row); xn in strip layout for sn
                xw_rb = xwpool.tile([BL, 1024], f16, tag="xw_rb", name="xw_rb")
                xw_zb = xwpool.tile([BL, 1024], f16, tag="xw_zb", name="xw_zb")
                xw_n = xwpool.tile([128, 256], f16, tag="xw_n", name="xw_n")
                nc.sync.dma_start(xw_rb[:], xw_dram[g][r0:r0 + BL, 0:1024])
                nc.sync.dma_start(xw_zb[:], xw_dram[g][r0:r0 + BL, 2048:3072])
                for j in range(NG):
                    nc.sync.dma_start(
                        xw_n[32 * j:32 * j + BL, :],
                        xw_dram[g][r0:r0 + BL, 1024 + 256 * j:1024 + 256 * (j + 1)])
                pmm_r = psum.tile([128, 256], f32, tag="mm_r", name="pmm_r",
                                  bufs=1)
                pmm_n = psum.tile([128, 256], f32, tag="mm_n", name="pmm_n",
                                  bufs=1)
                pmm_z = psum.tile([128, 256], f32, tag="mm_z", name="pmm_z",
                                  bufs=1)
                for j in range(NG):
                    nc.tensor.matmul(
                        pmm_r[32 * j:32 * j + BL, :],
                        ident_sb[0:BL, 0:BL],
                        xw_rb[:, 256 * j:256 * (j + 1)],
                        start=True, stop=False, tile_position=(0, 32 * j))
                for j in range(NG):
                    nc.tensor.matmul(
                        pmm_n[32 * j:32 * j + BL, :],
                        ones1_sb[0:1, :],
                        bhhn_sb[g][0:1, 256 * j:256 * (j + 1)],
                        start=True, stop=False, tile_position=(0, 32 * j))
                for j in range(NG):
                    nc.tensor.matmul(
                        pmm_z[32 * j:32 * j + BL, :],
                        ident_sb[0:BL, 0:BL],
                        xw_zb[:, 256 * j:256 * (j + 1)],
                        start=True, stop=False, tile_position=(0, 32 * j))
                return g, pmm_r, pmm_n, pmm_z, xw_n

            cur = stage_in(0)
            for step in range(2 * n_steps):
                g, pmm_r, pmm_n, pmm_z, xw_n = cur

                # three 256-col weight streams r -> n -> z: sig_r fires after
                # the first, the gate chain overlaps the rest. Even k-tiles
                # first: they read hT cols 0:128, produced by the first half
                # of the previous step's split transpose/hT-update.
                for c0, dst in ((0, pmm_r), (1024, pmm_n), (2048, pmm_z)):
                    for ki, k in enumerate((0, 2, 4, 6, 1, 3, 5, 7)):
                        coff = 128 * (k % 2) + 32 * (k // 2)
                        lhsT = hT[:, coff:coff + BL]
                        for j in range(NG):
                            nc.tensor.matmul(
                                dst[32 * j:32 * j + BL, :],
                                lhsT,
                                whh_sb[g][:, 3072 * k + c0 + 256 * j:
                                        3072 * k + c0 + 256 * (j + 1)],
                                start=False, stop=(ki == KT - 1),
                                tile_position=(0, 32 * j),
                            )

                # stage the next step now: its DMAs + PE preloads fill the
                # idle window between the z-stream and the transposes
                nxt = stage_in(step + 1) if step + 1 < 2 * n_steps else None

                # gates (strip view [128, *]; only partitions 32j+b<8 valid);
                # sigmoids read PSUM directly
                rz = wpool.tile([128, 512], f16, tag="rz")
                nc.scalar.activation(rz[:, 0:256], pmm_r[:], AF.Sigmoid)

                # n = tanh(xn + r * (hn + bhh_n)); pmm_n holds hn+bhh_n
                tn2 = wpool.tile([128, 256], f16, tag="tn2")
                nc.vector.tensor_mul(tn2[:], pmm_n[:], rz[:, 0:256])
                sn = wpool.tile([128, 256], f16, tag="sn")
                nc.vector.tensor_add(sn[:], tn2[:], xw_n[:])
                n_t = wpool.tile([128, 256], f16, tag="n_t")
                nc.scalar.activation(n_t[:], sn[:], AF.Tanh)
                nc.scalar.activation(rz[:, 256:512], pmm_z[:], AF.Sigmoid)

                # e = z'*(n-h) (z' mask-frozen); h updates by += e in both
                # layouts. Tail is split into column halves so hT cols 0:128
                # (even k-tiles) are ready before the second half finishes.
                d_t = wpool.tile([128, 256], f16, tag="d_t")
                nc.vector.tensor_sub(d_t[:], n_t[:], h_str[:])
                e_t = wpool.tile([128, 256], f16, tag="e_t")
                tp = psum.tile([128, 256], f16, tag="tp", bufs=1)
                hT_new = wpool.tile([128, 256], f16, tag="hT", name="hT_new")
                for c in range(2):
                    cs = slice(128 * c, 128 * (c + 1))
                    nc.vector.tensor_mul(e_t[:, cs], d_t[:, cs],
                                         rz[:, 256 + 128 * c:384 + 128 * c])
                    nc.tensor.transpose(tp[:, cs], e_t[:, cs], ident_sb[:])
                for c in range(2):
                    cs = slice(128 * c, 128 * (c + 1))
                    nc.vector.tensor_add(hT_new[:, cs], tp[:, cs], hT[:, cs])
                h_new = wpool.tile([128, 256], f16, tag="h_str", name="h_new")
                nc.gpsimd.tensor_add(h_new[:], e_t[:], h_str[:])

                h_str, hT = h_new, hT_new
                cur = nxt

            # ---- head ----------------------------------------------------
            ph = psum.tile([128, 512], f32, tag="p1", name="ph")
            for k in range(KT):
                coff = 128 * (k % 2) + 32 * (k // 2)
                nc.tensor.matmul(
                    ph[0:BL, 0:64],
                    hT[:, coff:coff + BL],
                    p1T_sb[:, 64 * k:64 * (k + 1)],
                    start=(k == 0), stop=(k == KT - 1),
                )
            t1s = wpool.tile([128, 64], f16, tag="t1s")
            nc.vector.tensor_add(t1s[0:BL, :], ph[0:BL, 0:64], p1b_sb[0:BL, :])
            t1 = wpool.tile([128, 64], f16, tag="t1")
            nc.scalar.activation(t1[0:BL, :], t1s[0:BL, :], AF.Tanh)

            pt1 = psum.tile([128, 256], f16, tag="tp", name="pt1", bufs=1)
            nc.tensor.transpose(pt1[0:64, 0:BL], t1[0:BL, 0:64], ident_sb[0:BL, 0:BL])
            t1T = wpool.tile([64, BL], f16, tag="t1T")
            nc.vector.tensor_copy(t1T[:], pt1[0:64, 0:BL])

            pl = psum.tile([128, 512], f32, tag="p1", name="pl")
            nc.tensor.matmul(pl[0:BL, 0:2], t1T[:], p2T_sb[:], start=True, stop=True)
            lg = wpool.tile([128, 2], f32, tag="lg")
            nc.vector.tensor_add(lg[0:BL, :], pl[0:BL, 0:2], p2b_sb[0:BL, :])
            nc.sync.dma_start(d_logits[:], lg[0:BL, :])

            if debug:
                nc.sync.dma_start(d_dbg_h[:], h_str[:])
                for g in ("src", "tgt"):
                    dbg = evpool.tile([128, 3072], f16, tag="dbgxw")
                    for mi in range(n_mstrip):
                        m0 = 128 * mi
                        msz = min(128, TB - m0)
                        nc.sync.dma_start(dbg[0:msz, :], xw_dram[g][m0:m0 + msz, :])
                        nc.sync.dma_start(d_dbg_xw[g][m0:m0 + msz, :], dbg[0:msz, :])

    nc.compile()
    return nc


# ----------------------------------------------------------------------------
# entry point
# ----------------------------------------------------------------------------

@functools.lru_cache(maxsize=2)
def _cached_program(n_steps, debug):
    return build_program(n_steps, debug)


def run(inputs, n_steps=T, debug=False, trace=False):
    inputs = {k: np.asarray(v) for k, v in inputs.items()}
    nc = _cached_program(n_steps, debug)
    shared = _prep_shared(inputs, n_steps)
    emb16 = np.asarray(inputs["emb"]).astype(np.float16)
    in_maps = []
    for c in range(NCORES):
        m = dict(shared)
        m.update(_prep_core(inputs, emb16, c, n_steps))
        in_maps.append(m)
    res = run_bass_kernel_spmd(nc, in_maps, list(range(NCORES)), trace=trace)
    logits = np.concatenate([res.results[c]["logits"] for c in range(NCORES)], axis=0)
    return logits, res


def kernel(**inputs) -> np.ndarray:
    logits, _ = run(inputs)
    return logits.astype(np.float32)

